# revision 33
# baseline (speedup 1.0000x reference)
"""Trainium2 Bass kernel for nn_Attn_69776038691596.

reference computes:
    proj     = einsum('bsh,kh->bsk', enc, W) + bias          # (B,S,H)
    energies = einsum('bh,bsh->bs', hid, proj)               # (B,S)
    out      = softmax(energies, axis=0)                     # over batch

Algebraic rewrite (exact in real arithmetic):
    u[b,:] = hid[b,:] @ W          # (B,H)  -- tiny matmul
    c[b]   = hid[b,:] . bias       # (B,)
    energies[b,s] = enc[b,s,:] . u[b,:] + c[b]

This turns a 275-GFLOP matmul into a 0.27-GFLOP weighted reduction that is
bound by reading encoder_output (512 MB) from HBM once.

Sharding: split the S axis (2048 -> 8 x 256) across the 8 cores. The softmax
runs over the batch axis, which every core holds entirely, so no collectives
are needed.

Per-core schedule (all contiguous HBM streams; gather layouts measured ~2x
slower on the HBM read side):
  - phase 0 (tiny): u = hid @ W on PE; c_row = bias . hid^T on PE;
    cb2[s,b] = c[b]/H broadcast via K=1 ones-matmul.
  - main loop over b: stream enc[b] as two contiguous (128s, H) tiles;
    broadcast u[b] over partitions via K=1 ones-matmul into PSUM;
    DVE multiplies, ScalarE activation-accumulates into E_half[s, b]
    (the c[b]/H activation bias folds in the energy offset).
  - softmax over the free (b) axis of E_half directly; PE-transpose the
    (128s, 64b) result to (64b, 128s) and stream out rows.
"""
import sys

sys.path.insert(0, "/opt/trn_rl_repo")

import numpy as np

B, S, H = 64, 2048, 1024
N_CORES = 8
S_LOC = S // N_CORES  # 256

_CACHE = {}


def build_nc(s_loc=S_LOC):
    """Build + compile the per-core Bass module. s_loc must be divisible by 128."""
    import concourse.bass as bass
    import concourse.bacc as bacc
    import concourse.tile as tile
    from concourse import mybir
    from concourse.masks import make_identity
    from contextlib import ExitStack

    f32 = mybir.dt.float32
    Alu = mybir.AluOpType
    Act = mybir.ActivationFunctionType
    X = mybir.AxisListType.X


    nc = bacc.Bacc("TRN2", target_bir_lowering=False, debug=False,
                   num_devices=N_CORES)
    enc = nc.dram_tensor("enc", [B, s_loc, H], f32, kind="ExternalInput").ap()
    hid = nc.dram_tensor("hid", [B, H], f32, kind="ExternalInput").ap()
    W = nc.dram_tensor("W", [H, H], f32, kind="ExternalInput").ap()
    bias = nc.dram_tensor("bias", [1, H], f32, kind="ExternalInput").ap()
    out = nc.dram_tensor("out", [B, s_loc], f32, kind="ExternalOutput").ap()

    with ExitStack() as ctx:
        tc = ctx.enter_context(tile.TileContext(nc))
        singles = ctx.enter_context(tc.tile_pool(name="singles", bufs=1))
        wpool = ctx.enter_context(tc.tile_pool(name="wpool", bufs=2))
        chunks = ctx.enter_context(tc.tile_pool(name="chunks", bufs=15))
        small = ctx.enter_context(tc.tile_pool(name="small", bufs=1))
        psum = ctx.enter_context(tc.tile_pool(name="psum", bufs=2, space="PSUM"))
        psum1 = ctx.enter_context(tc.tile_pool(name="psum1", bufs=1, space="PSUM"))

        # ---------- phase 0 ----------
        ident64 = singles.tile([64, 64], f32, tag="ident64")
        make_identity(nc, ident64)
        ident128 = singles.tile([128, 128], f32, tag="ident128")
        make_identity(nc, ident128)
        ones1 = singles.tile([1, 128], f32, tag="ones1")
        nc.vector.memset(ones1, 1.0)

        hid_sb = singles.tile([64, H], f32, tag="hid_sb")
        nc.sync.dma_start(out=hid_sb, in_=hid)

        # hidT[k] : (128k, 64b) via PE transpose
        hidT = []
        for k in range(8):
            pt = psum.tile([128, 64], f32, tag="pp")
            nc.tensor.transpose(pt, hid_sb[:, k * 128:(k + 1) * 128], ident64)
            st = singles.tile([128, 64], f32, tag=f"hidT_{k}")
            nc.vector.tensor_copy(st, pt)
            hidT.append(st)

        # bias as (128,1) per k-chunk; c_row = sum_k bias_k^T @ hidT_k : (1, 64)
        bias_sb = singles.tile([128, 8], f32, tag="bias_sb")
        nc.sync.dma_start(
            out=bias_sb,
            in_=bass.AP(tensor=bias.tensor, offset=bias.offset,
                        ap=[[1, 128], [128, 8]]))
        c_psum = psum.tile([1, 64], f32, tag="pp")
        for k in range(8):
            nc.tensor.matmul(c_psum, lhsT=bias_sb[:, k:k + 1], rhs=hidT[k],
                             start=(k == 0), stop=(k == 7))
        c_row = singles.tile([1, 64], f32, tag="c_row")
        nc.vector.tensor_scalar_mul(c_row, c_psum, 1.0 / H)

        # cb2[s, b] = c[b]/H on all 128 partitions (K=1 ones-matmul)
        cb_psum = psum.tile([128, 64], f32, tag="pp")
        nc.tensor.matmul(cb_psum, lhsT=ones1, rhs=c_row, start=True, stop=True)
        cb2 = singles.tile([128, 64], f32, tag="cb2")
        nc.vector.tensor_copy(cb2, cb_psum)

        # u = hid @ W : (64, H) via PE, accumulated over k in PSUM
        u_psum = psum1.tile([64, H], f32, tag="u_psum")
        for k in range(8):
            wk = wpool.tile([128, H], f32, tag="wk")
            nc.sync.dma_start(out=wk, in_=W[k * 128:(k + 1) * 128, :])
            for nh in range(2):
                nc.tensor.matmul(
                    u_psum[:, nh * 512:(nh + 1) * 512],
                    lhsT=hidT[k][:, 0:64],
                    rhs=wk[:, nh * 512:(nh + 1) * 512],
                    start=(k == 0), stop=(k == 7))
        u_sb = singles.tile([64, H], f32, tag="u_sb")
        nc.vector.tensor_copy(u_sb, u_psum)

        # Split u into 3 bf16 terms (hi+mid+lo carries ~24 mantissa bits, so
        # the bf16 PE broadcast below reconstructs u to fp32 accuracy).
        bf16 = mybir.dt.bfloat16
        usplit = singles.tile([64, 3, H], bf16, tag="usplit")
        r1 = singles.tile([64, H], f32, tag="r1")
        r2 = singles.tile([64, H], f32, tag="r2")
        nc.vector.tensor_copy(usplit[:, 0, :], u_sb)
        nc.vector.tensor_sub(r1, u_sb, usplit[:, 0, :])
        nc.vector.tensor_copy(usplit[:, 1, :], r1)
        nc.vector.tensor_sub(r2, r1, usplit[:, 1, :])
        nc.vector.tensor_copy(usplit[:, 2, :], r2)
        ones3 = singles.tile([3, 128], bf16, tag="ones3")
        nc.vector.memset(ones3, 1.0)

        # ---------- phase 1: energies ----------
        # Tile for batch b: partition p = s//2, free (r = s%2, h); one 1 MB
        # DMA per b with 8 KB-per-partition contiguous descriptors.
        # Er[r][p, b] = energy(b, s = 2p + r).
        pp = s_loc // 2
        Eh = [singles.tile([pp, B], f32, tag=f"E{i}", name=f"E{i}")
              for i in range(2)]
        encv = enc.rearrange("b (p two) h -> b p (two h)", two=2)
        psumB = ctx.enter_context(tc.tile_pool(name="psumB", bufs=2, space="PSUM"))
        stgpool = ctx.enter_context(tc.tile_pool(name="stgpool", bufs=3))
        for b in range(B):
            # ub[s, h] = u[b, h] broadcast into PSUM: stage the 3 bf16 split
            # rows of u[b] onto partitions 0-2 (tiny DMA), then one K=3
            # bf16 ones-matmul per 512-wide half sums hi+mid+lo on all 128
            # partitions. (An fp32 PE broadcast streams at ~9 cycles/column
            # and paces the kernel at ~240us; a GpSimd broadcast contends
            # with DVE for SBUF ports and slows the muls 1.7x.)
            stg = stgpool.tile([3, H], bf16, tag="stg")
            nc.sync.dma_start(out=stg, in_=usplit[b:b + 1, :, :])
            ub = psumB.tile([128, H], f32, tag="ub")
            for nh in range(2):
                nc.tensor.matmul(ub[:, nh * 512:(nh + 1) * 512],
                                 lhsT=ones3,
                                 rhs=stg[:, nh * 512:(nh + 1) * 512],
                                 start=True, stop=True)
            ck = chunks.tile([pp, 2, H], f32, tag="ck")
            nc.sync.dma_start(out=ck, in_=encv[b])
            for r in range(2):
                nc.vector.tensor_mul(ck[:, r, :], ck[:, r, :], ub[0:pp, :])
                # ScalarE: E[p, b] = sum_h(ck + c[b]/H) = enc[b,2p+r,:].u + c[b]
                nc.scalar.activation(ck[:, r, :], ck[:, r, :], Act.Identity,
                                     bias=cb2[0:pp, b:b + 1], scale=1.0,
                                     accum_out=Eh[r][:, b:b + 1])

        # ---------- phase 2: softmax over b (free axis), emit out ----------
        identPP = ident128 if pp == 128 else ident64
        O = small.tile([64, pp, 2], f32, tag="O")
        for r in range(2):
            e = Eh[r]
            negm = small.tile([pp, 1], f32, tag=f"negm{r}")
            nc.vector.tensor_reduce(negm, e, axis=X, op=Alu.max, negate=True)
            ssum = small.tile([pp, 1], f32, tag=f"ssum{r}")
            nc.scalar.activation(e, e, Act.Exp, bias=negm, scale=1.0,
                                 accum_out=ssum)
            rs = small.tile([pp, 1], f32, tag=f"rs{r}")
            nc.vector.reciprocal(rs, ssum)
            nc.vector.tensor_scalar_mul(e, e, rs)
            # transpose (pp s', 64b) -> (64b, pp s'), interleave r
            op = psum.tile([64, pp], f32, tag="pp")
            nc.tensor.transpose(op, e, identPP)
            nc.vector.tensor_copy(O[:, :, r], op)
        outv = out.rearrange("b (p r) -> b p r", r=2)
        nc.sync.dma_start(out=outv, in_=O)

    nc.compile()
    return nc


def _get_nc():
    if "nc" not in _CACHE:
        _CACHE["nc"] = build_nc()
    return _CACHE["nc"]


def run_spmd(hidden, encoder_output, W, b, **spmd_kwargs):
    from concourse.bass_utils import run_bass_kernel_spmd

    nc = _get_nc()
    hid2d = np.ascontiguousarray(np.asarray(hidden, dtype=np.float32)[0])
    Wn = np.ascontiguousarray(np.asarray(W, dtype=np.float32))
    bn = np.ascontiguousarray(np.asarray(b, dtype=np.float32).reshape(1, H))
    enc = np.asarray(encoder_output, dtype=np.float32)
    in_maps = []
    for c in range(N_CORES):
        in_maps.append({
            "enc": np.ascontiguousarray(enc[:, c * S_LOC:(c + 1) * S_LOC, :]),
            "hid": hid2d,
            "W": Wn,
            "bias": bn,
        })
    return run_bass_kernel_spmd(nc, in_maps, core_ids=list(range(N_CORES)),
                                **spmd_kwargs)


def kernel(hidden, encoder_output, W, b):
    res = run_spmd(hidden, encoder_output, W, b)
    return np.concatenate([res.results[c]["out"] for c in range(N_CORES)], axis=1)


# revision 37
# speedup vs baseline: 1.0980x; 1.0980x over previous
"""Trainium2 Bass kernel for nn_Attn_69776038691596.

reference computes:
    proj     = einsum('bsh,kh->bsk', enc, W) + bias          # (B,S,H)
    energies = einsum('bh,bsh->bs', hid, proj)               # (B,S)
    out      = softmax(energies, axis=0)                     # over batch

Algebraic rewrite (exact in real arithmetic):
    u[b,:] = hid[b,:] @ W          # (B,H)  -- tiny matmul
    c[b]   = hid[b,:] . bias       # (B,)
    energies[b,s] = enc[b,s,:] . u[b,:] + c[b]

This turns a 275-GFLOP matmul into a 0.27-GFLOP weighted reduction that is
bound by reading encoder_output (512 MB) from HBM once.

Sharding: split the S axis (2048 -> 8 x 256) across the 8 cores. The softmax
runs over the batch axis, which every core holds entirely, so no collectives
are needed.

Per-core schedule (all contiguous HBM streams; gather layouts measured ~2x
slower on the HBM read side):
  - phase 0 (tiny): u = hid @ W on PE (fp32); u split into 3 exact bf16
    terms; c_row = bias . hid^T on PE; cb2[s,b] = c[b]/H broadcast via a
    K=1 ones-matmul. All 8 W tiles get their own SBUF slot so no W load
    ever waits on PE -- a waiting DMA head-of-line-blocks the whole HWDGE
    ring and stalls the enc prefetch (measured ~20us).
  - main loop over b: one 1 MB DMA streams enc[b] into a (128, 2, H) tile
    (partition p = s//2, 8 KB contiguous per partition); u[b] is broadcast
    into PSUM by a K=3 bf16 ones-matmul over its split rows (fp32 PE
    streaming is ~9 cyc/col and would pace the kernel; GpSimd broadcast
    contends with DVE on SBUF ports); DVE multiplies; ScalarE
    activation-accumulates into Er[s%2][s//2, b] with the c[b]/H bias
    folding in the energy offset.
  - softmax over the free (b) axis of each Er directly; PE-transpose the
    (128, 64b) results, DVE-interleave r, one output DMA.

Measured on 8 axon trn2 cores: ~217-245 us HW exec (DMA-bound; enc bytes /
358 GB/s = 179 us is the hard floor, engine busy: DMA ~212, DVE ~165,
ACT ~165, PE ~89).
"""
import sys

sys.path.insert(0, "/opt/trn_rl_repo")

import numpy as np

B, S, H = 64, 2048, 1024
N_CORES = 8
S_LOC = S // N_CORES  # 256

_CACHE = {}


def build_nc(s_loc=S_LOC):
    """Build + compile the per-core Bass module. s_loc must be divisible by 128."""
    import concourse.bass as bass
    import concourse.bacc as bacc
    import concourse.tile as tile
    from concourse import mybir
    from concourse.masks import make_identity
    from contextlib import ExitStack

    f32 = mybir.dt.float32
    Alu = mybir.AluOpType
    Act = mybir.ActivationFunctionType
    X = mybir.AxisListType.X


    nc = bacc.Bacc("TRN2", target_bir_lowering=False, debug=False,
                   num_devices=N_CORES)
    enc = nc.dram_tensor("enc", [B, s_loc, H], f32, kind="ExternalInput").ap()
    hid = nc.dram_tensor("hid", [B, H], f32, kind="ExternalInput").ap()
    W = nc.dram_tensor("W", [H, H], f32, kind="ExternalInput").ap()
    bias = nc.dram_tensor("bias", [1, H], f32, kind="ExternalInput").ap()
    out = nc.dram_tensor("out", [B, s_loc], f32, kind="ExternalOutput").ap()

    with ExitStack() as ctx:
        tc = ctx.enter_context(tile.TileContext(nc))
        singles = ctx.enter_context(tc.tile_pool(name="singles", bufs=1))
        wpool = ctx.enter_context(tc.tile_pool(name="wpool", bufs=8))
        chunks = ctx.enter_context(tc.tile_pool(name="chunks", bufs=15))
        small = ctx.enter_context(tc.tile_pool(name="small", bufs=1))
        psum = ctx.enter_context(tc.tile_pool(name="psum", bufs=2, space="PSUM"))
        psum1 = ctx.enter_context(tc.tile_pool(name="psum1", bufs=1, space="PSUM"))

        # ---------- phase 0 ----------
        ident64 = singles.tile([64, 64], f32, tag="ident64")
        make_identity(nc, ident64)
        ident128 = singles.tile([128, 128], f32, tag="ident128")
        make_identity(nc, ident128)
        ones1 = singles.tile([1, 128], f32, tag="ones1")
        nc.vector.memset(ones1, 1.0)

        hid_sb = singles.tile([64, H], f32, tag="hid_sb")
        nc.sync.dma_start(out=hid_sb, in_=hid)

        # hidT[k] : (128k, 64b) via PE transpose
        hidT = []
        for k in range(8):
            pt = psum.tile([128, 64], f32, tag="pp")
            nc.tensor.transpose(pt, hid_sb[:, k * 128:(k + 1) * 128], ident64)
            st = singles.tile([128, 64], f32, tag=f"hidT_{k}")
            nc.vector.tensor_copy(st, pt)
            hidT.append(st)

        # u = hid @ W : (64, H) via PE, accumulated over k in PSUM
        u_psum = psum1.tile([64, H], f32, tag="u_psum")
        for k in range(8):
            wk = wpool.tile([128, H], f32, tag="wk")
            nc.sync.dma_start(out=wk, in_=W[k * 128:(k + 1) * 128, :])
            for nh in range(2):
                nc.tensor.matmul(
                    u_psum[:, nh * 512:(nh + 1) * 512],
                    lhsT=hidT[k][:, 0:64],
                    rhs=wk[:, nh * 512:(nh + 1) * 512],
                    start=(k == 0), stop=(k == 7))
        u_sb = singles.tile([64, H], f32, tag="u_sb")
        nc.vector.tensor_copy(u_sb, u_psum)

        # bias as (128,1) per k-chunk; c_row = sum_k bias_k^T @ hidT_k : (1, 64)
        bias_sb = singles.tile([128, 8], f32, tag="bias_sb")
        nc.sync.dma_start(
            out=bias_sb,
            in_=bass.AP(tensor=bias.tensor, offset=bias.offset,
                        ap=[[1, 128], [128, 8]]))
        c_psum = psum.tile([1, 64], f32, tag="pp")
        for k in range(8):
            nc.tensor.matmul(c_psum, lhsT=bias_sb[:, k:k + 1], rhs=hidT[k],
                             start=(k == 0), stop=(k == 7))
        c_row = singles.tile([1, 64], f32, tag="c_row")
        nc.vector.tensor_scalar_mul(c_row, c_psum, 1.0 / H)

        # cb2[s, b] = c[b]/H on all 128 partitions (K=1 ones-matmul)
        cb_psum = psum.tile([128, 64], f32, tag="pp")
        nc.tensor.matmul(cb_psum, lhsT=ones1, rhs=c_row, start=True, stop=True)
        cb2 = singles.tile([128, 64], f32, tag="cb2")
        nc.vector.tensor_copy(cb2, cb_psum)


        # Split u into 3 bf16 terms (hi+mid+lo carries ~24 mantissa bits, so
        # the bf16 PE broadcast below reconstructs u to fp32 accuracy).
        bf16 = mybir.dt.bfloat16
        usplit = singles.tile([64, 3, H], bf16, tag="usplit")
        r1 = singles.tile([64, H], f32, tag="r1")
        r2 = singles.tile([64, H], f32, tag="r2")
        nc.vector.tensor_copy(usplit[:, 0, :], u_sb)
        nc.vector.tensor_sub(r1, u_sb, usplit[:, 0, :])
        nc.vector.tensor_copy(usplit[:, 1, :], r1)
        nc.vector.tensor_sub(r2, r1, usplit[:, 1, :])
        nc.vector.tensor_copy(usplit[:, 2, :], r2)
        ones3 = singles.tile([3, 128], bf16, tag="ones3")
        nc.vector.memset(ones3, 1.0)

        # ---------- phase 1: energies ----------
        # Tile for batch b: partition p = s//2, free (r = s%2, h); one 1 MB
        # DMA per b with 8 KB-per-partition contiguous descriptors.
        # Er[r][p, b] = energy(b, s = 2p + r).
        pp = s_loc // 2
        Eh = [singles.tile([pp, B], f32, tag=f"E{i}", name=f"E{i}")
              for i in range(2)]
        encv = enc.rearrange("b (p two) h -> b p (two h)", two=2)
        psumB = ctx.enter_context(tc.tile_pool(name="psumB", bufs=2, space="PSUM"))
        stgpool = ctx.enter_context(tc.tile_pool(name="stgpool", bufs=3))
        for b in range(B):
            # ub[s, h] = u[b, h] broadcast into PSUM: stage the 3 bf16 split
            # rows of u[b] onto partitions 0-2 (tiny DMA), then one K=3
            # bf16 ones-matmul per 512-wide half sums hi+mid+lo on all 128
            # partitions. (An fp32 PE broadcast streams at ~9 cycles/column
            # and paces the kernel at ~240us; a GpSimd broadcast contends
            # with DVE for SBUF ports and slows the muls 1.7x.)
            stg = stgpool.tile([3, H], bf16, tag="stg")
            nc.sync.dma_start(out=stg, in_=usplit[b:b + 1, :, :])
            ub = psumB.tile([128, H], f32, tag="ub")
            for nh in range(2):
                nc.tensor.matmul(ub[:, nh * 512:(nh + 1) * 512],
                                 lhsT=ones3,
                                 rhs=stg[:, nh * 512:(nh + 1) * 512],
                                 start=True, stop=True)
            ck = chunks.tile([pp, 2, H], f32, tag="ck")
            nc.sync.dma_start(out=ck, in_=encv[b])
            for r in range(2):
                nc.vector.tensor_mul(ck[:, r, :], ck[:, r, :], ub[0:pp, :])
                # ScalarE: E[p, b] = sum_h(ck + c[b]/H) = enc[b,2p+r,:].u + c[b]
                nc.scalar.activation(ck[:, r, :], ck[:, r, :], Act.Identity,
                                     bias=cb2[0:pp, b:b + 1], scale=1.0,
                                     accum_out=Eh[r][:, b:b + 1])

        # ---------- phase 2: softmax over b (free axis), emit out ----------
        identPP = ident128 if pp == 128 else ident64
        O = small.tile([64, pp, 2], f32, tag="O")
        for r in range(2):
            e = Eh[r]
            negm = small.tile([pp, 1], f32, tag=f"negm{r}")
            nc.vector.tensor_reduce(negm, e, axis=X, op=Alu.max, negate=True)
            ssum = small.tile([pp, 1], f32, tag=f"ssum{r}")
            nc.scalar.activation(e, e, Act.Exp, bias=negm, scale=1.0,
                                 accum_out=ssum)
            rs = small.tile([pp, 1], f32, tag=f"rs{r}")
            nc.vector.reciprocal(rs, ssum)
            nc.vector.tensor_scalar_mul(e, e, rs)
            # transpose (pp s', 64b) -> (64b, pp s'), interleave r
            op = psum.tile([64, pp], f32, tag="pp")
            nc.tensor.transpose(op, e, identPP)
            nc.vector.tensor_copy(O[:, :, r], op)
        outv = out.rearrange("b (p r) -> b p r", r=2)
        nc.sync.dma_start(out=outv, in_=O)

    nc.compile()
    return nc


def _get_nc():
    if "nc" not in _CACHE:
        _CACHE["nc"] = build_nc()
    return _CACHE["nc"]


def run_spmd(hidden, encoder_output, W, b, **spmd_kwargs):
    from concourse.bass_utils import run_bass_kernel_spmd

    nc = _get_nc()
    hid2d = np.ascontiguousarray(np.asarray(hidden, dtype=np.float32)[0])
    Wn = np.ascontiguousarray(np.asarray(W, dtype=np.float32))
    bn = np.ascontiguousarray(np.asarray(b, dtype=np.float32).reshape(1, H))
    enc = np.asarray(encoder_output, dtype=np.float32)
    in_maps = []
    for c in range(N_CORES):
        in_maps.append({
            "enc": np.ascontiguousarray(enc[:, c * S_LOC:(c + 1) * S_LOC, :]),
            "hid": hid2d,
            "W": Wn,
            "bias": bn,
        })
    return run_bass_kernel_spmd(nc, in_maps, core_ids=list(range(N_CORES)),
                                **spmd_kwargs)


def kernel(hidden, encoder_output, W, b):
    res = run_spmd(hidden, encoder_output, W, b)
    return np.concatenate([res.results[c]["out"] for c in range(N_CORES)], axis=1)


# revision 38
# speedup vs baseline: 1.1558x; 1.0526x over previous
"""Trainium2 Bass kernel for nn_Attn_69776038691596.

reference computes:
    proj     = einsum('bsh,kh->bsk', enc, W) + bias          # (B,S,H)
    energies = einsum('bh,bsh->bs', hid, proj)               # (B,S)
    out      = softmax(energies, axis=0)                     # over batch

Algebraic rewrite (exact in real arithmetic):
    u[b,:] = hid[b,:] @ W          # (B,H)  -- tiny matmul
    c[b]   = hid[b,:] . bias       # (B,)
    energies[b,s] = enc[b,s,:] . u[b,:] + c[b]

This turns a 275-GFLOP matmul into a 0.27-GFLOP weighted reduction that is
bound by reading encoder_output (512 MB) from HBM once.

Sharding: split the S axis (2048 -> 8 x 256) across the 8 cores. The softmax
runs over the batch axis, which every core holds entirely, so no collectives
are needed.

Per-core schedule (all contiguous HBM streams; gather layouts measured ~2x
slower on the HBM read side):
  - phase 0 (tiny): u = hid @ W on PE (fp32); u split into 3 exact bf16
    terms; c_row = bias . hid^T on PE; cb2[s,b] = c[b]/H broadcast via a
    K=1 ones-matmul. All 8 W tiles get their own SBUF slot so no W load
    ever waits on PE -- a waiting DMA head-of-line-blocks the whole HWDGE
    ring and stalls the enc prefetch (measured ~20us).
  - main loop over b: one 1 MB DMA streams enc[b] into a (128, 2, H) tile
    (partition p = s//2, 8 KB contiguous per partition); u[b] is broadcast
    into PSUM by a K=3 bf16 ones-matmul over its split rows (fp32 PE
    streaming is ~9 cyc/col and would pace the kernel; GpSimd broadcast
    contends with DVE on SBUF ports); DVE multiplies; ScalarE
    activation-accumulates into Er[s%2][s//2, b] with the c[b]/H bias
    folding in the energy offset.
  - softmax over the free (b) axis of each Er directly; PE-transpose the
    (128, 64b) results, DVE-interleave r, one output DMA.

Measured on 8 axon trn2 cores: ~217-245 us HW exec (DMA-bound; enc bytes /
358 GB/s = 179 us is the hard floor, engine busy: DMA ~212, DVE ~165,
ACT ~165, PE ~89).
"""
import sys

sys.path.insert(0, "/opt/trn_rl_repo")

import numpy as np

B, S, H = 64, 2048, 1024
N_CORES = 8
S_LOC = S // N_CORES  # 256

_CACHE = {}


def build_nc(s_loc=S_LOC):
    """Build + compile the per-core Bass module. s_loc must be divisible by 128."""
    import concourse.bass as bass
    import concourse.bacc as bacc
    import concourse.tile as tile
    from concourse import mybir
    from concourse.masks import make_identity
    from contextlib import ExitStack

    f32 = mybir.dt.float32
    Alu = mybir.AluOpType
    Act = mybir.ActivationFunctionType
    X = mybir.AxisListType.X


    nc = bacc.Bacc("TRN2", target_bir_lowering=False, debug=False,
                   num_devices=N_CORES)
    enc = nc.dram_tensor("enc", [B, s_loc, H], f32, kind="ExternalInput").ap()
    hid = nc.dram_tensor("hid", [B, H], f32, kind="ExternalInput").ap()
    W = nc.dram_tensor("W", [H, H], f32, kind="ExternalInput").ap()
    bias = nc.dram_tensor("bias", [1, H], f32, kind="ExternalInput").ap()
    out = nc.dram_tensor("out", [B, s_loc], f32, kind="ExternalOutput").ap()

    with ExitStack() as ctx:
        tc = ctx.enter_context(tile.TileContext(nc))
        singles = ctx.enter_context(tc.tile_pool(name="singles", bufs=1))
        wpool = ctx.enter_context(tc.tile_pool(name="wpool", bufs=8))
        chunks = ctx.enter_context(tc.tile_pool(name="chunks", bufs=15))
        small = ctx.enter_context(tc.tile_pool(name="small", bufs=1))
        psum = ctx.enter_context(tc.tile_pool(name="psum", bufs=2, space="PSUM"))
        psum1 = ctx.enter_context(tc.tile_pool(name="psum1", bufs=1, space="PSUM"))

        # ---------- phase 0 ----------
        ident64 = singles.tile([64, 64], f32, tag="ident64")
        make_identity(nc, ident64)
        ident128 = singles.tile([128, 128], f32, tag="ident128")
        make_identity(nc, ident128)
        ones1 = singles.tile([1, 128], f32, tag="ones1")
        nc.vector.memset(ones1, 1.0)

        hid_sb = singles.tile([64, H], f32, tag="hid_sb")
        nc.sync.dma_start(out=hid_sb, in_=hid)

        # hidT[k] : (128k, 64b) via PE transpose
        hidT = []
        for k in range(8):
            pt = psum.tile([128, 64], f32, tag="pp")
            nc.tensor.transpose(pt, hid_sb[:, k * 128:(k + 1) * 128], ident64)
            st = singles.tile([128, 64], f32, tag=f"hidT_{k}")
            nc.vector.tensor_copy(st, pt)
            hidT.append(st)

        # u = hid @ W : (64, H) via PE, accumulated over k in PSUM
        u_psum = psum1.tile([64, H], f32, tag="u_psum")
        for k in range(8):
            wk = wpool.tile([128, H], f32, tag="wk")
            nc.sync.dma_start(out=wk, in_=W[k * 128:(k + 1) * 128, :])
            for nh in range(2):
                nc.tensor.matmul(
                    u_psum[:, nh * 512:(nh + 1) * 512],
                    lhsT=hidT[k][:, 0:64],
                    rhs=wk[:, nh * 512:(nh + 1) * 512],
                    start=(k == 0), stop=(k == 7))
        u_sb = singles.tile([64, H], f32, tag="u_sb")
        nc.vector.tensor_copy(u_sb, u_psum)

        # bias as (128,1) per k-chunk; c_row = sum_k bias_k^T @ hidT_k : (1, 64)
        bias_sb = singles.tile([128, 8], f32, tag="bias_sb")
        nc.sync.dma_start(
            out=bias_sb,
            in_=bass.AP(tensor=bias.tensor, offset=bias.offset,
                        ap=[[1, 128], [128, 8]]))
        c_psum = psum.tile([1, 64], f32, tag="pp")
        for k in range(8):
            nc.tensor.matmul(c_psum, lhsT=bias_sb[:, k:k + 1], rhs=hidT[k],
                             start=(k == 0), stop=(k == 7))
        c_row = singles.tile([1, 64], f32, tag="c_row")
        nc.vector.tensor_scalar_mul(c_row, c_psum, 1.0 / H)

        # cb2[s, b] = c[b]/H on all 128 partitions (K=1 ones-matmul)
        cb_psum = psum.tile([128, 64], f32, tag="pp")
        nc.tensor.matmul(cb_psum, lhsT=ones1, rhs=c_row, start=True, stop=True)
        cb2 = singles.tile([128, 64], f32, tag="cb2")
        nc.vector.tensor_copy(cb2, cb_psum)


        # Split u into 3 bf16 terms (hi+mid+lo carries ~24 mantissa bits, so
        # the bf16 PE broadcast below reconstructs u to fp32 accuracy).
        bf16 = mybir.dt.bfloat16
        usplit = singles.tile([64, 3, H], bf16, tag="usplit")
        r1 = singles.tile([64, H], f32, tag="r1")
        r2 = singles.tile([64, H], f32, tag="r2")
        nc.vector.tensor_copy(usplit[:, 0, :], u_sb)
        nc.vector.tensor_sub(r1, u_sb, usplit[:, 0, :])
        nc.vector.tensor_copy(usplit[:, 1, :], r1)
        nc.vector.tensor_sub(r2, r1, usplit[:, 1, :])
        nc.vector.tensor_copy(usplit[:, 2, :], r2)
        ones3 = singles.tile([3, 128], bf16, tag="ones3")
        nc.vector.memset(ones3, 1.0)

        # ---------- phase 1: energies ----------
        # Tile for batch b: partition p = s//2, free (r = s%2, h); one 1 MB
        # DMA per b with 8 KB-per-partition contiguous descriptors.
        # Er[r][p, b] = energy(b, s = 2p + r).
        pp = s_loc // 2
        Eh = [singles.tile([pp, B], f32, tag=f"E{i}", name=f"E{i}")
              for i in range(2)]
        encv = enc.rearrange("b (p two) h -> b p (two h)", two=2)
        psumB = ctx.enter_context(tc.tile_pool(name="psumB", bufs=2, space="PSUM"))
        stgpool = ctx.enter_context(tc.tile_pool(name="stgpool", bufs=3))
        # Issue the first PREFETCH enc DMAs ahead of any stg DMA in ring
        # order: stg[0] waits on the DVE usplit chain, and a waiting DMA
        # head-of-line-blocks the FIFO HWDGE ring (would stall all prefetch).
        PREFETCH = min(12, B)
        cks = []
        for b in range(PREFETCH):
            ck = chunks.tile([pp, 2, H], f32, tag="ck", name=f"ckp{b}")
            nc.sync.dma_start(out=ck, in_=encv[b])
            cks.append(ck)
        for b in range(B):
            # ub[s, h] = u[b, h] broadcast into PSUM: stage the 3 bf16 split
            # rows of u[b] onto partitions 0-2 (tiny DMA), then one K=3
            # bf16 ones-matmul per 512-wide half sums hi+mid+lo on all 128
            # partitions. (An fp32 PE broadcast streams at ~9 cycles/column
            # and paces the kernel at ~240us; a GpSimd broadcast contends
            # with DVE for SBUF ports and slows the muls 1.7x.)
            stg = stgpool.tile([3, H], bf16, tag="stg")
            nc.sync.dma_start(out=stg, in_=usplit[b:b + 1, :, :])
            ub = psumB.tile([128, H], f32, tag="ub")
            for nh in range(2):
                nc.tensor.matmul(ub[:, nh * 512:(nh + 1) * 512],
                                 lhsT=ones3,
                                 rhs=stg[:, nh * 512:(nh + 1) * 512],
                                 start=True, stop=True)
            if b < PREFETCH:
                ck = cks[b]
            else:
                ck = chunks.tile([pp, 2, H], f32, tag="ck")
                nc.sync.dma_start(out=ck, in_=encv[b])
            for r in range(2):
                nc.vector.tensor_mul(ck[:, r, :], ck[:, r, :], ub[0:pp, :])
                # ScalarE: E[p, b] = sum_h(ck + c[b]/H) = enc[b,2p+r,:].u + c[b]
                nc.scalar.activation(ck[:, r, :], ck[:, r, :], Act.Identity,
                                     bias=cb2[0:pp, b:b + 1], scale=1.0,
                                     accum_out=Eh[r][:, b:b + 1])

        # ---------- phase 2: softmax over b (free axis), emit out ----------
        identPP = ident128 if pp == 128 else ident64
        O = small.tile([64, pp, 2], f32, tag="O")
        for r in range(2):
            e = Eh[r]
            negm = small.tile([pp, 1], f32, tag=f"negm{r}")
            nc.vector.tensor_reduce(negm, e, axis=X, op=Alu.max, negate=True)
            ssum = small.tile([pp, 1], f32, tag=f"ssum{r}")
            nc.scalar.activation(e, e, Act.Exp, bias=negm, scale=1.0,
                                 accum_out=ssum)
            rs = small.tile([pp, 1], f32, tag=f"rs{r}")
            nc.vector.reciprocal(rs, ssum)
            nc.vector.tensor_scalar_mul(e, e, rs)
            # transpose (pp s', 64b) -> (64b, pp s'), interleave r
            op = psum.tile([64, pp], f32, tag="pp")
            nc.tensor.transpose(op, e, identPP)
            nc.vector.tensor_copy(O[:, :, r], op)
        outv = out.rearrange("b (p r) -> b p r", r=2)
        nc.sync.dma_start(out=outv, in_=O)

    nc.compile()
    return nc


def _get_nc():
    if "nc" not in _CACHE:
        _CACHE["nc"] = build_nc()
    return _CACHE["nc"]


def run_spmd(hidden, encoder_output, W, b, **spmd_kwargs):
    from concourse.bass_utils import run_bass_kernel_spmd

    nc = _get_nc()
    hid2d = np.ascontiguousarray(np.asarray(hidden, dtype=np.float32)[0])
    Wn = np.ascontiguousarray(np.asarray(W, dtype=np.float32))
    bn = np.ascontiguousarray(np.asarray(b, dtype=np.float32).reshape(1, H))
    enc = np.asarray(encoder_output, dtype=np.float32)
    in_maps = []
    for c in range(N_CORES):
        in_maps.append({
            "enc": np.ascontiguousarray(enc[:, c * S_LOC:(c + 1) * S_LOC, :]),
            "hid": hid2d,
            "W": Wn,
            "bias": bn,
        })
    return run_bass_kernel_spmd(nc, in_maps, core_ids=list(range(N_CORES)),
                                **spmd_kwargs)


def kernel(hidden, encoder_output, W, b):
    res = run_spmd(hidden, encoder_output, W, b)
    return np.concatenate([res.results[c]["out"] for c in range(N_CORES)], axis=1)
